# revision 26
# baseline (speedup 1.0000x reference)
"""Distributed SimCLR/NT-Xent contrastive loss on 8 Trainium2 NeuronCores.

Symmetric-half scheme: the 2Bx2B similarity matrix is symmetric, so each
core computes only ~half of its row-block's exp() elements:

  * rotated inputs put core c's own rows at column group 0; groups g map to
    global row-blocks (c+g) mod 8.
  * per 128-row strip m, the computed run is: g0 cols [128m, 2048) +
    g1,g2,g3 full + g4 cols [128m, 2048)   (133120 cols vs 262144 full).
  * row sums come from ACT Exp accum_out; the transposed (uncomputed)
    halves are recovered as COLUMN sums of the computed exp tiles, done on
    the PE as tiny matmuls (lhsT = exp chunk [128,128], rhs = ones [128,1],
    out = [128,1] PSUM accumulated across strips via start/stop).
  * the diagonal 128x128 sub-blocks of g0/g4 are covered by both the row
    pass and the column pass; their row sums (DVE reduce over the exp tile)
    are subtracted once. Both sides compute bit-identical values, so the
    correction is exact.
  * each core DMAs out raw partial row sums, column sums, and positive-pair
    dots; the host assembles full softmax denominators across cores and
    takes log/mean there (16K logs, trivial).
  * the ET build (normalize + transpose per block) is interleaved with the
    main strip/piece loop: a piece is emitted as soon as every column group
    it reads has been built. Matmul outputs are bf16 so two 2048-wide psum
    piece tiles + the colsum bank + the transpose scratch all fit in PSUM.

loss = 1/T + mean_i log(S_i) - mean_i dot(a_i, b_i)/T
(true diagonal masked via a -1e9*I matmul before exp, as in the reference)
"""

import sys

if "/opt/trn_rl_repo" not in sys.path:
    sys.path.insert(0, "/opt/trn_rl_repo")

import numpy as np

import concourse.bass as bass
import concourse.mybir as mybir
from concourse import masks
from concourse.tile import TileContext
from concourse.bass_utils import run_bass_kernel_spmd

# ---------------------------------------------------------------------------
# Compatibility patches for the walrus build in this container:
#  * EVENT_SEMAPHORE_RANGE_CLEAR fails codegen ("ISA wrong length"), and
#  * the tile teardown Drain carries >2 sem waits ("Too many sync wait
#    commands").
# Replace the teardown with per-proc single-wait drains + barriers and skip
# the on-device semaphore clear (allocator bookkeeping is kept).
# ---------------------------------------------------------------------------


def _patched_clear_and_free_semaphores(self, sems):
    if not sems:
        return
    sem_nums = [
        s.num if isinstance(s, bass.SemaphoreHandle) else s for s in sems
    ]
    self._state.prepend_free_semaphores(sem_nums)
    for poison_set in self._tile_sem_poison_stack:
        poison_set.update(sem_nums)


def _patched_drain_and_barrier(self, tick_clock, wait_clock):
    nc = self.nc
    clock = tick_clock.global_clock
    assert self.sems is not None
    allocated = self.sems.allocated()  # proc index -> SemaphoreHandle
    for proc in sorted(allocated):
        sem = allocated[proc]
        tick = clock[proc]
        if tick <= 0:
            continue
        mult = 16 if sem.name.startswith("DMA") else 1
        d = nc.sync.drain()
        d.wait_op(sem, tick * mult, "sem-ge")
    nc.all_engine_barrier()
    popped = nc._tile_sem_poison_stack.pop()
    assert popped is self._sem_poison
    nc.clear_and_free_semaphores(list(allocated.values()))
    nc.all_engine_barrier()


bass.Bass.clear_and_free_semaphores = _patched_clear_and_free_semaphores
TileContext._drain_and_barrier = _patched_drain_and_barrier


def _hoist_excess_waits(nc, limit=1):
    """This walrus supports only `limit` sync waits per instruction. Hoist
    the excess onto standalone EventSemaphore instructions inserted just
    before the over-subscribed instruction on the same engine (per-engine
    program order makes this semantically identical)."""
    import bass_rust

    counter = 0
    for bb in nc.main_func.blocks:
        insts = bb.instructions
        new = []
        changed = False
        for ins in insts:
            si = ins.sync_info
            if si is not None:
                waits = list(si.on_wait)
                if len(waits) > limit:
                    excess, keep = waits[:-limit], waits[-limit:]
                    for w in excess:
                        counter += 1
                        ev = mybir.InstEventSemaphore(
                            name=f"hoistw-{counter}",
                            engine=ins.engine,
                            ins=[],
                            outs=[],
                        )
                        ev.sync_info = bass_rust.SyncInfo(
                            on_wait=[w], on_update=[]
                        )
                        new.append(ev)
                    ins.sync_info = bass_rust.SyncInfo(
                        on_wait=keep, on_update=list(si.on_update)
                    )
                    changed = True
            new.append(ins)
        if changed:
            bb.instructions = new

TEMPERATURE = 0.07
B, D = 8192, 128
N2 = 2 * B
NCORES = 8
P = 128

F32 = mybir.dt.float32
F32R = mybir.dt.float32r
BF16 = mybir.dt.bfloat16
AF = mybir.ActivationFunctionType
ALU = mybir.AluOpType
AX = mybir.AxisListType

CAP = 1536     # psum piece width (f32 -> 3 banks per piece tile)
NGRP_CS = 5    # groups 0..4 produce column sums


def _strip_runs(m, bs=2048):
    return [
        (0, P * m, bs), (1, 0, bs), (2, 0, bs), (3, 0, bs), (4, P * m, bs)
    ]


def _run_lookup(runs, off):
    """Map run-offset -> (group, group-col)."""
    for g, gs, ge in runs:
        if off < ge - gs:
            return g, gs + off
        off -= ge - gs
    raise AssertionError("offset out of run")


def _build_bass(n2=N2, ncores=NCORES, mm_dtype=BF16, hoist=True):
    """Build the per-core SPMD program (symmetric-half scheme)."""
    scale = 1.0 / TEMPERATURE
    rpc = n2 // ncores          # rows per core (2048)
    mt = rpc // P               # 128-row strips per core (16)
    bs = rpc                    # column-group width (2048)
    nb = n2 // bs               # number of groups (8)
    jb = bs // P                # rows packed per partition in DMA (16)
    half = n2 // 2
    pb = half // bs             # partner block index (4)
    assert half % bs == 0

    nbb = NGRP_CS                # blocks actually consumed (groups 0..4)
    nc = bass.Bass()
    allx = nc.dram_tensor("allx", [nbb * rpc, D], BF16, kind="ExternalInput")
    # out columns: [0:16] stot_net, [16:96] colsums (5 groups x 16 chunks),
    # [96:112] positive-pair dots
    OUTW = mt + NGRP_CS * jb + jb
    out = nc.dram_tensor("out", [P, OUTW], F32, kind="ExternalOutput")

    # packed DMA view: row = b*bs + p*jb + j -> [block b, partition p, slot j]
    allx_b = allx[:].rearrange("(b p j) d -> b p (j d)", p=P, j=jb)

    # static strip-major piece list: (strip m, piece idx, base, width,
    # n_pieces of strip, max group needed)
    pieces_all = []
    for m in range(mt):
        runs = _strip_runs(m, bs)
        W = sum(ge - gs for _, gs, ge in runs)
        npieces = (W + CAP - 1) // CAP
        for pi in range(npieces):
            base = pi * CAP
            w = min(CAP, W - base)
            maxg = _run_lookup(runs, base + w - 1)[0]
            pieces_all.append((m, pi, base, w, npieces, maxg))

    with TileContext(nc) as tc:
        with (
            tc.tile_pool(name="persist", bufs=1) as persist,
            tc.tile_pool(name="rawx", bufs=3) as rawx_pool,
            tc.tile_pool(name="raw0", bufs=1) as raw0_pool,
            tc.tile_pool(name="xn", bufs=2) as xn_pool,
            tc.tile_pool(name="sq", bufs=2) as sq_pool,
            tc.tile_pool(name="parts", bufs=2) as parts_pool,
            tc.tile_pool(name="exps", bufs=4) as exps_pool,
            tc.tile_pool(name="psAB", bufs=2, space="PSUM") as psAB_pool,
            tc.tile_pool(name="psCS", bufs=1, space="PSUM") as psCS_pool,
            tc.tile_pool(name="bpsum", bufs=1, space="PSUM") as bpsum,
        ):
            ident = persist.tile([P, P], BF16, tag="ident")
            masks.make_identity(nc, ident[:])
            bias_negs = persist.tile([P, 1], F32, tag="bias_negs")
            nc.gpsimd.memset(bias_negs[:], -scale)
            ones_bf = persist.tile([P, 1], BF16, tag="ones_bf")
            nc.gpsimd.memset(ones_bf[:], 1.0)
            # -1e9 * I, accumulated into the diag chunk by a tiny PE matmul
            # (keeps the self-sim mask off the DVE/ACT critical path)
            negid = persist.tile([P, P], BF16, tag="negid")
            nc.gpsimd.memset(negid[:], 0.0)
            nc.gpsimd.affine_select(
                out=negid[:], in_=negid[:],
                compare_op=ALU.not_equal, fill=-1.0e9,
                base=0, pattern=[[-1, P]], channel_multiplier=1,
            )

            et = [
                persist.tile([P, bs], mm_dtype, tag=f"et{gi}", name=f"et{gi}")
                for gi in range(nbb)
            ]
            norms2 = persist.tile([P, nbb * jb], F32, tag="norms2")
            rsq = persist.tile([P, nbb * jb], BF16, tag="rsq")
            lntmp = persist.tile([P, nbb * jb], F32, tag="lntmp")
            rawdot = persist.tile([P, jb], F32, tag="rawdot")
            pos2 = persist.tile([P, jb], F32, tag="pos2")
            pospart = persist.tile([P, jb], F32, tag="pospart")
            stot = persist.tile([P, mt], F32, tag="stot")
            corr0 = persist.tile([P, mt], F32, tag="corr0")
            corr4 = persist.tile([P, mt], F32, tag="corr4")
            out_sb = persist.tile([P, OUTW], F32, tag="out_sb")
            cs_sb = persist.tile([P, NGRP_CS * jb], F32, tag="cs_sb")

            # one full bank so later psum pool allocations stay bank-aligned
            cs = psCS_pool.tile([P, 512], F32, tag="cs")

            # colsum matmuls for piece i are emitted after the mains of
            # piece i+1: the PE pre-fills the next psum tile before
            # stalling on ACT(i), keeping ACT continuously fed.
            deferred = []

            def emit_colsums(ex, runs, m, pbase, pw):
                for k in range(pw // P):
                    g, gcol = _run_lookup(runs, pbase + k * P)
                    j = gcol // P
                    last = mt - 1 if g in (1, 2, 3) else j
                    slot = g * jb + j
                    nc.tensor.matmul(
                        cs[:, slot : slot + 1],
                        ex[:, k * P : (k + 1) * P],
                        ones_bf[:],
                        start=(m == 0),
                        stop=(m == last),
                        skip_group_check=True,
                    )

            strip_partials = {}

            def emit_piece(m, pi, pbase, pw, npieces, maxg):
                runs = _strip_runs(m, bs)
                lhsT = et[0][:, m * P : (m + 1) * P]
                if pi == 0:
                    strip_partials[m] = parts_pool.tile(
                        [P, npieces], F32, name="partials"
                    )
                partials = strip_partials[m]
                ps = psAB_pool.tile([P, CAP], F32, tag="psAB")
                off = 0
                first = True
                while off < pw:
                    g, gcol = _run_lookup(runs, pbase + off)
                    _, gs, ge = runs[g]
                    wmax = min(512 - (off % 512), ge - gcol, pw - off)
                    nc.tensor.matmul(
                        ps[:, off : off + wmax],
                        lhsT,
                        et[g][:, gcol : gcol + wmax],
                        start=True,
                        stop=True,
                    )
                    off += wmax
                    if first and pi == 0:
                        # self-similarity diag sits in the first 128 cols:
                        # accumulate -1e9*I on top of the first chunk
                        nc.tensor.matmul(
                            ps[:, 0:P],
                            negid[:],
                            ident[:],
                            start=False,
                            stop=True,
                            skip_group_check=True,
                        )
                    first = False
                # flush the previous piece's deferred colsums now that this
                # piece's mains are queued on the PE
                while deferred:
                    emit_colsums(*deferred.pop(0))
                ex = exps_pool.tile([P, CAP], BF16, name="ex")
                nc.scalar.activation(
                    ex[:, :pw], ps[:, :pw], AF.Exp,
                    bias=bias_negs[:], scale=scale,
                    accum_out=partials[:, pi : pi + 1],
                )
                deferred.append((ex, runs, m, pbase, pw))
                # diag-subblock corrections (double-count removal)
                if pi == 0:
                    nc.vector.reduce_sum(
                        corr0[:, m : m + 1],
                        ex[:, 0:P].rearrange("p (a x) -> p a x", a=1),
                        axis=AX.X,
                    )
                g4off = (bs - P * m) + 3 * bs  # run-offset of g4 diag chunk
                if pbase <= g4off < pbase + pw:
                    o = g4off - pbase
                    nc.vector.reduce_sum(
                        corr4[:, m : m + 1],
                        ex[:, o : o + P].rearrange("p (a x) -> p a x", a=1),
                        axis=AX.X,
                    )
                if pi == npieces - 1:
                    nc.vector.reduce_sum(
                        stot[:, m : m + 1], partials[:], axis=AX.X
                    )
                    del strip_partials[m]

            # ---- interleaved build + main loop --------------------------
            qi = 0
            raw_blocks = {}
            for b in range(nbb):
                pool = raw0_pool if b in (0, pb) else rawx_pool
                rx = pool.tile(
                    [P, bs], BF16, tag=f"raw{b}" if b in (0, pb) else ""
                )
                nc.sync.dma_start(rx[:], allx_b[b])
                raw_blocks[b] = rx
                rx3 = rx[:].rearrange("p (j d) -> p j d", d=D)
                js = slice(b * jb, (b + 1) * jb)
                sq = sq_pool.tile([P, bs], BF16)
                nc.vector.tensor_mul(sq[:], rx[:], rx[:])
                nc.vector.reduce_sum(
                    norms2[:, js],
                    sq[:].rearrange("p (j d) -> p j d", d=D),
                    axis=AX.X,
                )
                # rsqrt(x) = exp(-0.5*ln(x)); Ln+Exp share one table set
                nc.scalar.activation(lntmp[:, js], norms2[:, js], AF.Ln)
                nc.scalar.activation(
                    rsq[:, js], lntmp[:, js], AF.Exp, scale=-0.5
                )
                xn = xn_pool.tile([P, bs], BF16)
                nc.vector.tensor_mul(
                    xn[:].rearrange("p (j d) -> p j d", d=D),
                    rx3,
                    rsq[:, js].to_broadcast((P, jb, D)),
                )
                xn3 = xn[:].rearrange("p (j d) -> p j d", d=D)
                # transposes in two half-block batches through a 1-bank
                # bf16 psum scratch; scatter back to natural row order:
                # et col p*jb + j <- ps col (j - 8h)*P + p
                hjb = jb // 2
                for h in range(2):
                    ps = bpsum.tile([P, hjb * P], BF16, tag="bps")
                    for j in range(hjb):
                        nc.tensor.transpose(
                            ps[:, j * P : (j + 1) * P],
                            xn3[:, h * hjb + j, :],
                            ident[:],
                        )
                    nc.vector.tensor_copy(
                        et[b][:].rearrange("q (p j) -> q p j", j=jb)[
                            :, :, h * hjb : (h + 1) * hjb
                        ],
                        ps[:].rearrange("q (j p) -> q p j", p=P),
                    )
                if b == pb:
                    # positive-pair raw dots: my rows x partner rows
                    r0 = raw_blocks[0][:].rearrange("p (j d) -> p j d", d=D)
                    rp = raw_blocks[pb][:].rearrange("p (j d) -> p j d", d=D)
                    pd = sq_pool.tile([P, bs], F32)
                    pd3 = pd[:].rearrange("p (j d) -> p j d", d=D)
                    nc.vector.tensor_mul(pd3[:], r0[:], rp[:])
                    nc.vector.reduce_sum(rawdot[:], pd3[:], axis=AX.X)
                    nc.vector.tensor_mul(pos2[:], rawdot[:], rsq[:, 0:jb])
                    nc.vector.tensor_mul(
                        pospart[:], pos2[:],
                        rsq[:, pb * jb : (pb + 1) * jb],
                    )
                # emit every piece whose groups are all built
                while qi < len(pieces_all) and pieces_all[qi][5] <= b:
                    emit_piece(*pieces_all[qi])
                    qi += 1
            while qi < len(pieces_all):
                emit_piece(*pieces_all[qi])
                qi += 1

            # ---- tail ------------------------------------------------
            while deferred:
                emit_colsums(*deferred.pop(0))
            nc.scalar.copy(cs_sb[:], cs[:, : NGRP_CS * jb])

            nc.vector.tensor_sub(stot[:], stot[:], corr0[:])
            nc.vector.tensor_sub(out_sb[:, 0:mt], stot[:], corr4[:])
            nc.vector.tensor_copy(
                out_sb[:, mt : mt + NGRP_CS * jb], cs_sb[:]
            )
            nc.vector.tensor_copy(
                out_sb[:, mt + NGRP_CS * jb : OUTW], pospart[:]
            )
            nc.sync.dma_start(out[:], out_sb[:])

    if hoist:
        _hoist_excess_waits(nc, limit=1)
    return nc


def _in_maps(embeddings_a, embeddings_b, ncores=NCORES):
    import ml_dtypes

    allx = np.ascontiguousarray(
        np.concatenate([embeddings_a, embeddings_b], axis=0)
    ).astype(ml_dtypes.bfloat16)
    n2 = allx.shape[0]
    rpc = n2 // ncores
    maps = []
    for c in range(ncores):
        # rotate so this core's rows sit at column group 0; only groups
        # 0..4 are consumed on-device
        rot = np.ascontiguousarray(
            np.roll(allx, -c * rpc, axis=0)[: NGRP_CS * rpc]
        )
        maps.append({"allx": rot})
    return maps


def _combine(outs, n2=N2, ncores=NCORES):
    """outs: list of per-core [P, OUTW] partials -> scalar loss (f32).

    Assembles full softmax denominators: own row sums + own cs0 + cs_g
    from cores (c-g) mod 8 for g=1..4, then log/mean on the host.
    """
    mt = n2 // ncores // P        # 16
    jb = mt                       # 16 chunks per group
    inv_t = 1.0 / TEMPERATURE

    stot = np.zeros((ncores, n2 // ncores), dtype=np.float64)
    cs = np.zeros((NGRP_CS, ncores, n2 // ncores), dtype=np.float64)
    pos_sum = 0.0
    for c, o in enumerate(outs):
        o64 = np.asarray(o, dtype=np.float64)
        # [p, m] -> row 128m + p
        stot[c] = o64[:, 0:mt].T.reshape(-1)
        for g in range(NGRP_CS):
            blk = o64[:, mt + g * jb : mt + (g + 1) * jb]
            cs[g, c] = blk.T.reshape(-1)
        pos_sum += o64[:, mt + NGRP_CS * jb :].sum()

    S = stot + cs[0]
    for g in range(1, NGRP_CS):
        for c in range(ncores):
            S[c] += cs[g, (c - g) % ncores]

    loss = inv_t + np.log(S).mean() - pos_sum * inv_t / n2
    return np.float32(loss)


_NC_CACHE = {}


def _get_nc():
    if "nc" not in _NC_CACHE:
        _NC_CACHE["nc"] = _build_bass()
    return _NC_CACHE["nc"]


def kernel(embeddings_a, embeddings_b):
    nc = _get_nc()
    maps = _in_maps(embeddings_a, embeddings_b)
    res = run_bass_kernel_spmd(nc, maps, list(range(NCORES)), trace=False)
    return _combine([r["out"] for r in res.results])


# revision 30
# speedup vs baseline: 1.0002x; 1.0002x over previous
"""Distributed SimCLR/NT-Xent contrastive loss on 8 Trainium2 NeuronCores.

Symmetric-half scheme: the 2Bx2B similarity matrix is symmetric, so each
core computes only ~half of its row-block's exp() elements:

  * rotated inputs put core c's own rows at column group 0; groups g map to
    global row-blocks (c+g) mod 8.
  * per 128-row strip m, the computed run is: g0 cols [128m, 2048) +
    g1,g2,g3 full + g4 cols [128m, 2048)   (133120 cols vs 262144 full).
  * row sums come from ACT Exp accum_out; the transposed (uncomputed)
    halves are recovered as COLUMN sums of the computed exp tiles, done on
    the PE as tiny matmuls (lhsT = exp chunk [128,128], rhs = ones [128,1],
    out = [128,1] PSUM accumulated across strips via start/stop).
  * the diagonal 128x128 sub-blocks of g0/g4 are covered by both the row
    pass and the column pass; their row sums (DVE reduce over the exp tile)
    are subtracted once. Both sides compute bit-identical values, so the
    correction is exact.
  * each core DMAs out raw partial row sums, column sums, and positive-pair
    dots; the host assembles full softmax denominators across cores and
    takes log/mean there (16K logs, trivial).
  * the ET build (normalize + transpose per block) is interleaved with the
    main strip/piece loop: a piece is emitted as soon as every column group
    it reads has been built. Matmul outputs are bf16 so two 2048-wide psum
    piece tiles + the colsum bank + the transpose scratch all fit in PSUM.

loss = 1/T + mean_i log(S_i) - mean_i dot(a_i, b_i)/T
(true diagonal masked via a -1e9*I matmul before exp, as in the reference)
"""

import sys

if "/opt/trn_rl_repo" not in sys.path:
    sys.path.insert(0, "/opt/trn_rl_repo")

import numpy as np

import concourse.bass as bass
import concourse.mybir as mybir
from concourse import masks
from concourse.tile import TileContext
from concourse.bass_utils import run_bass_kernel_spmd

# ---------------------------------------------------------------------------
# Compatibility patches for the walrus build in this container:
#  * EVENT_SEMAPHORE_RANGE_CLEAR fails codegen ("ISA wrong length"), and
#  * the tile teardown Drain carries >2 sem waits ("Too many sync wait
#    commands").
# Replace the teardown with per-proc single-wait drains + barriers and skip
# the on-device semaphore clear (allocator bookkeeping is kept).
# ---------------------------------------------------------------------------


def _patched_clear_and_free_semaphores(self, sems):
    if not sems:
        return
    sem_nums = [
        s.num if isinstance(s, bass.SemaphoreHandle) else s for s in sems
    ]
    self._state.prepend_free_semaphores(sem_nums)
    for poison_set in self._tile_sem_poison_stack:
        poison_set.update(sem_nums)


def _patched_drain_and_barrier(self, tick_clock, wait_clock):
    nc = self.nc
    clock = tick_clock.global_clock
    assert self.sems is not None
    allocated = self.sems.allocated()  # proc index -> SemaphoreHandle
    for proc in sorted(allocated):
        sem = allocated[proc]
        tick = clock[proc]
        if tick <= 0:
            continue
        mult = 16 if sem.name.startswith("DMA") else 1
        d = nc.sync.drain()
        d.wait_op(sem, tick * mult, "sem-ge")
    nc.all_engine_barrier()
    popped = nc._tile_sem_poison_stack.pop()
    assert popped is self._sem_poison
    nc.clear_and_free_semaphores(list(allocated.values()))
    nc.all_engine_barrier()


bass.Bass.clear_and_free_semaphores = _patched_clear_and_free_semaphores
TileContext._drain_and_barrier = _patched_drain_and_barrier


def _hoist_excess_waits(nc, limit=1):
    """This walrus supports only `limit` sync waits per instruction. Hoist
    the excess onto standalone EventSemaphore instructions inserted just
    before the over-subscribed instruction on the same engine (per-engine
    program order makes this semantically identical)."""
    import bass_rust

    counter = 0
    for bb in nc.main_func.blocks:
        insts = bb.instructions
        new = []
        changed = False
        for ins in insts:
            si = ins.sync_info
            if si is not None:
                waits = list(si.on_wait)
                if len(waits) > limit:
                    excess, keep = waits[:-limit], waits[-limit:]
                    for w in excess:
                        counter += 1
                        ev = mybir.InstEventSemaphore(
                            name=f"hoistw-{counter}",
                            engine=ins.engine,
                            ins=[],
                            outs=[],
                        )
                        ev.sync_info = bass_rust.SyncInfo(
                            on_wait=[w], on_update=[]
                        )
                        new.append(ev)
                    ins.sync_info = bass_rust.SyncInfo(
                        on_wait=keep, on_update=list(si.on_update)
                    )
                    changed = True
            new.append(ins)
        if changed:
            bb.instructions = new

TEMPERATURE = 0.07
B, D = 8192, 128
N2 = 2 * B
NCORES = 8
P = 128

F32 = mybir.dt.float32
F32R = mybir.dt.float32r
BF16 = mybir.dt.bfloat16
AF = mybir.ActivationFunctionType
ALU = mybir.AluOpType
AX = mybir.AxisListType

CAP = 1536     # psum piece width (f32 -> 3 banks per piece tile)
NGRP_CS = 5    # groups 0..4 produce column sums


def _strip_runs(m, bs=2048):
    return [
        (0, P * m, bs), (1, 0, bs), (2, 0, bs), (3, 0, bs), (4, P * m, bs)
    ]


def _run_lookup(runs, off):
    """Map run-offset -> (group, group-col)."""
    for g, gs, ge in runs:
        if off < ge - gs:
            return g, gs + off
        off -= ge - gs
    raise AssertionError("offset out of run")


def _build_bass(n2=N2, ncores=NCORES, mm_dtype=BF16, hoist=True):
    """Build the per-core SPMD program (symmetric-half scheme)."""
    scale = 1.0 / TEMPERATURE
    rpc = n2 // ncores          # rows per core (2048)
    mt = rpc // P               # 128-row strips per core (16)
    bs = rpc                    # column-group width (2048)
    nb = n2 // bs               # number of groups (8)
    jb = bs // P                # rows packed per partition in DMA (16)
    half = n2 // 2
    pb = half // bs             # partner block index (4)
    assert half % bs == 0

    nbb = NGRP_CS                # blocks actually consumed (groups 0..4)
    nc = bass.Bass()
    allx = nc.dram_tensor("allx", [nbb * rpc, D], BF16, kind="ExternalInput")
    # out columns: [0:16] stot_net, [16:96] colsums (5 groups x 16 chunks),
    # [96:112] positive-pair dots
    OUTW = mt + NGRP_CS * jb + jb
    out = nc.dram_tensor("out", [P, OUTW], F32, kind="ExternalOutput")

    # packed DMA view: row = b*bs + p*jb + j -> [block b, partition p, slot j]
    allx_b = allx[:].rearrange("(b p j) d -> b p (j d)", p=P, j=jb)

    # static strip-major piece list: (strip m, piece idx, base, width,
    # n_pieces of strip, max group needed)
    pieces_all = []
    for m in range(mt):
        runs = _strip_runs(m, bs)
        W = sum(ge - gs for _, gs, ge in runs)
        npieces = (W + CAP - 1) // CAP
        for pi in range(npieces):
            base = pi * CAP
            w = min(CAP, W - base)
            maxg = _run_lookup(runs, base + w - 1)[0]
            pieces_all.append((m, pi, base, w, npieces, maxg))

    with TileContext(nc) as tc:
        with (
            tc.tile_pool(name="persist", bufs=1) as persist,
            tc.tile_pool(name="rawx", bufs=3) as rawx_pool,
            tc.tile_pool(name="raw0", bufs=1) as raw0_pool,
            tc.tile_pool(name="xn", bufs=2) as xn_pool,
            tc.tile_pool(name="sq", bufs=2) as sq_pool,
            tc.tile_pool(name="parts", bufs=2) as parts_pool,
            tc.tile_pool(name="exps", bufs=4) as exps_pool,
            tc.tile_pool(name="psAB", bufs=2, space="PSUM") as psAB_pool,
            tc.tile_pool(name="psCS", bufs=1, space="PSUM") as psCS_pool,
            tc.tile_pool(name="bpsum", bufs=1, space="PSUM") as bpsum,
        ):
            ident = persist.tile([P, P], BF16, tag="ident")
            masks.make_identity(nc, ident[:])
            bias_negs = persist.tile([P, 1], F32, tag="bias_negs")
            nc.gpsimd.memset(bias_negs[:], -scale)
            ones_bf = persist.tile([P, 1], BF16, tag="ones_bf")
            nc.gpsimd.memset(ones_bf[:], 1.0)
            zerosT = persist.tile([P, P], BF16, tag="zerosT")
            nc.gpsimd.memset(zerosT[:], 0.0)
            # -1e9 * I, accumulated into the diag chunk by a tiny PE matmul
            # (keeps the self-sim mask off the DVE/ACT critical path)
            negid = persist.tile([P, P], BF16, tag="negid")
            nc.gpsimd.memset(negid[:], 0.0)
            nc.gpsimd.affine_select(
                out=negid[:], in_=negid[:],
                compare_op=ALU.not_equal, fill=-1.0e9,
                base=0, pattern=[[-1, P]], channel_multiplier=1,
            )

            et = [
                persist.tile([P, bs], mm_dtype, tag=f"et{gi}", name=f"et{gi}")
                for gi in range(nbb)
            ]
            norms2 = persist.tile([P, nbb * jb], F32, tag="norms2")
            rsq = persist.tile([P, nbb * jb], BF16, tag="rsq")
            lntmp = persist.tile([P, nbb * jb], F32, tag="lntmp")
            rawdot = persist.tile([P, jb], F32, tag="rawdot")
            pos2 = persist.tile([P, jb], F32, tag="pos2")
            pospart = persist.tile([P, jb], F32, tag="pospart")
            stot = persist.tile([P, mt], F32, tag="stot")
            corr0 = persist.tile([P, mt], F32, tag="corr0")
            corr4 = persist.tile([P, mt], F32, tag="corr4")
            out_sb = persist.tile([P, OUTW], F32, tag="out_sb")
            cs_sb = persist.tile([P, NGRP_CS * jb], F32, tag="cs_sb")

            # one full bank so later psum pool allocations stay bank-aligned
            cs = psCS_pool.tile([P, 512], F32, tag="cs")

            # colsum matmuls for piece i are emitted after the mains of
            # piece i+1: the PE pre-fills the next psum tile before
            # stalling on ACT(i), keeping ACT continuously fed.
            deferred = []

            def emit_colsums(ex, runs, m, pbase, pw):
                for k in range(pw // P):
                    g, gcol = _run_lookup(runs, pbase + k * P)
                    j = gcol // P
                    last = mt - 1 if g in (1, 2, 3) else j
                    slot = g * jb + j
                    # pure accumulate: the cs bank was zeroed by one
                    # bank-wide start=True matmul (start=True here would
                    # re-mark the whole 2KB zero-region pending-zero and
                    # wipe other slots' partials)
                    nc.tensor.matmul(
                        cs[:, slot : slot + 1],
                        ex[:, k * P : (k + 1) * P],
                        ones_bf[:],
                        start=False,
                        stop=False,
                        skip_group_check=True,
                    )

            strip_partials = {}

            def emit_piece(m, pi, pbase, pw, npieces, maxg):
                runs = _strip_runs(m, bs)
                lhsT = et[0][:, m * P : (m + 1) * P]
                if pi == 0:
                    strip_partials[m] = parts_pool.tile(
                        [P, npieces], F32, name="partials"
                    )
                partials = strip_partials[m]
                ps = psAB_pool.tile([P, CAP], F32, tag="psAB")
                off = 0
                first = True
                while off < pw:
                    g, gcol = _run_lookup(runs, pbase + off)
                    _, gs, ge = runs[g]
                    wmax = min(512 - (off % 512), ge - gcol, pw - off)
                    nc.tensor.matmul(
                        ps[:, off : off + wmax],
                        lhsT,
                        et[g][:, gcol : gcol + wmax],
                        start=True,
                        stop=True,
                    )
                    off += wmax
                    if first and pi == 0:
                        # self-similarity diag sits in the first 128 cols:
                        # accumulate -1e9*I on top of the first chunk
                        nc.tensor.matmul(
                            ps[:, 0:P],
                            negid[:],
                            ident[:],
                            start=False,
                            stop=True,
                            skip_group_check=True,
                        )
                    first = False
                # flush the previous piece's deferred colsums now that this
                # piece's mains are queued on the PE
                while deferred:
                    emit_colsums(*deferred.pop(0))
                ex = exps_pool.tile([P, CAP], BF16, name="ex")
                nc.scalar.activation(
                    ex[:, :pw], ps[:, :pw], AF.Exp,
                    bias=bias_negs[:], scale=scale,
                    accum_out=partials[:, pi : pi + 1],
                )
                deferred.append((ex, runs, m, pbase, pw))
                # diag-subblock corrections (double-count removal)
                if pi == 0:
                    nc.vector.reduce_sum(
                        corr0[:, m : m + 1],
                        ex[:, 0:P].rearrange("p (a x) -> p a x", a=1),
                        axis=AX.X,
                    )
                g4off = (bs - P * m) + 3 * bs  # run-offset of g4 diag chunk
                if pbase <= g4off < pbase + pw:
                    o = g4off - pbase
                    nc.vector.reduce_sum(
                        corr4[:, m : m + 1],
                        ex[:, o : o + P].rearrange("p (a x) -> p a x", a=1),
                        axis=AX.X,
                    )
                if pi == npieces - 1:
                    nc.vector.reduce_sum(
                        stot[:, m : m + 1], partials[:], axis=AX.X
                    )
                    del strip_partials[m]

            # ---- interleaved build + main loop --------------------------
            qi = 0
            raw_blocks = {}
            for b in range(nbb):
                pool = raw0_pool if b in (0, pb) else rawx_pool
                rx = pool.tile(
                    [P, bs], BF16, tag=f"raw{b}" if b in (0, pb) else ""
                )
                nc.sync.dma_start(rx[:], allx_b[b])
                raw_blocks[b] = rx
                if b == 0:
                    # zero + open the whole colsum bank in one matmul
                    nc.tensor.matmul(
                        cs[:], zerosT[:], rx[:, 0:512],
                        start=True, stop=False, skip_group_check=True,
                    )
                rx3 = rx[:].rearrange("p (j d) -> p j d", d=D)
                js = slice(b * jb, (b + 1) * jb)
                sq = sq_pool.tile([P, bs], BF16)
                nc.vector.tensor_mul(sq[:], rx[:], rx[:])
                nc.vector.reduce_sum(
                    norms2[:, js],
                    sq[:].rearrange("p (j d) -> p j d", d=D),
                    axis=AX.X,
                )
                # rsqrt(x) = exp(-0.5*ln(x)); Ln+Exp share one table set
                nc.scalar.activation(lntmp[:, js], norms2[:, js], AF.Ln)
                nc.scalar.activation(
                    rsq[:, js], lntmp[:, js], AF.Exp, scale=-0.5
                )
                xn = xn_pool.tile([P, bs], BF16)
                nc.vector.tensor_mul(
                    xn[:].rearrange("p (j d) -> p j d", d=D),
                    rx3,
                    rsq[:, js].to_broadcast((P, jb, D)),
                )
                xn3 = xn[:].rearrange("p (j d) -> p j d", d=D)
                # transposes in two half-block batches through a 1-bank
                # bf16 psum scratch; scatter back to natural row order:
                # et col p*jb + j <- ps col (j - 8h)*P + p
                hjb = jb // 2
                for h in range(2):
                    ps = bpsum.tile([P, hjb * P], BF16, tag="bps")
                    for j in range(hjb):
                        nc.tensor.transpose(
                            ps[:, j * P : (j + 1) * P],
                            xn3[:, h * hjb + j, :],
                            ident[:],
                        )
                    nc.vector.tensor_copy(
                        et[b][:].rearrange("q (p j) -> q p j", j=jb)[
                            :, :, h * hjb : (h + 1) * hjb
                        ],
                        ps[:].rearrange("q (j p) -> q p j", p=P),
                    )
                if b == pb:
                    # positive-pair raw dots: my rows x partner rows
                    r0 = raw_blocks[0][:].rearrange("p (j d) -> p j d", d=D)
                    rp = raw_blocks[pb][:].rearrange("p (j d) -> p j d", d=D)
                    pd = sq_pool.tile([P, bs], F32)
                    pd3 = pd[:].rearrange("p (j d) -> p j d", d=D)
                    nc.vector.tensor_mul(pd3[:], r0[:], rp[:])
                    nc.vector.reduce_sum(rawdot[:], pd3[:], axis=AX.X)
                    nc.vector.tensor_mul(pos2[:], rawdot[:], rsq[:, 0:jb])
                    nc.vector.tensor_mul(
                        pospart[:], pos2[:],
                        rsq[:, pb * jb : (pb + 1) * jb],
                    )
                # emit every piece whose groups are all built
                while qi < len(pieces_all) and pieces_all[qi][5] <= b:
                    emit_piece(*pieces_all[qi])
                    qi += 1
            while qi < len(pieces_all):
                emit_piece(*pieces_all[qi])
                qi += 1

            # ---- tail ------------------------------------------------
            while deferred:
                emit_colsums(*deferred.pop(0))
            # close the colsum accumulation group
            nc.tensor.matmul(
                cs[:], zerosT[:], raw_blocks[0][:, 0:512],
                start=False, stop=True, skip_group_check=True,
            )
            nc.scalar.copy(cs_sb[:], cs[:, : NGRP_CS * jb])

            nc.vector.tensor_sub(stot[:], stot[:], corr0[:])
            nc.vector.tensor_sub(out_sb[:, 0:mt], stot[:], corr4[:])
            nc.vector.tensor_copy(
                out_sb[:, mt : mt + NGRP_CS * jb], cs_sb[:]
            )
            nc.vector.tensor_copy(
                out_sb[:, mt + NGRP_CS * jb : OUTW], pospart[:]
            )
            nc.sync.dma_start(out[:], out_sb[:])

    if hoist:
        _hoist_excess_waits(nc, limit=1)
    return nc


def _in_maps(embeddings_a, embeddings_b, ncores=NCORES):
    import ml_dtypes

    allx = np.ascontiguousarray(
        np.concatenate([embeddings_a, embeddings_b], axis=0)
    ).astype(ml_dtypes.bfloat16)
    n2 = allx.shape[0]
    rpc = n2 // ncores
    maps = []
    for c in range(ncores):
        # rotate so this core's rows sit at column group 0; only groups
        # 0..4 are consumed on-device
        rot = np.ascontiguousarray(
            np.roll(allx, -c * rpc, axis=0)[: NGRP_CS * rpc]
        )
        maps.append({"allx": rot})
    return maps


def _combine(outs, n2=N2, ncores=NCORES):
    """outs: list of per-core [P, OUTW] partials -> scalar loss (f32).

    Assembles full softmax denominators: own row sums + own cs0 + cs_g
    from cores (c-g) mod 8 for g=1..4, then log/mean on the host.
    """
    mt = n2 // ncores // P        # 16
    jb = mt                       # 16 chunks per group
    inv_t = 1.0 / TEMPERATURE

    stot = np.zeros((ncores, n2 // ncores), dtype=np.float64)
    cs = np.zeros((NGRP_CS, ncores, n2 // ncores), dtype=np.float64)
    pos_sum = 0.0
    for c, o in enumerate(outs):
        o64 = np.asarray(o, dtype=np.float64)
        # [p, m] -> row 128m + p
        stot[c] = o64[:, 0:mt].T.reshape(-1)
        for g in range(NGRP_CS):
            blk = o64[:, mt + g * jb : mt + (g + 1) * jb]
            cs[g, c] = blk.T.reshape(-1)
        pos_sum += o64[:, mt + NGRP_CS * jb :].sum()

    S = stot + cs[0]
    for g in range(1, NGRP_CS):
        for c in range(ncores):
            S[c] += cs[g, (c - g) % ncores]

    loss = inv_t + np.log(S).mean() - pos_sum * inv_t / n2
    return np.float32(loss)


_NC_CACHE = {}


def _get_nc():
    if "nc" not in _NC_CACHE:
        _NC_CACHE["nc"] = _build_bass()
    return _NC_CACHE["nc"]


def kernel(embeddings_a, embeddings_b):
    nc = _get_nc()
    maps = _in_maps(embeddings_a, embeddings_b)
    res = run_bass_kernel_spmd(nc, maps, list(range(NCORES)), trace=False)
    return _combine([r["out"] for r in res.results])
